# revision 14
# baseline (speedup 1.0000x reference)
"""Causal full-d_model attention (B=4, T=2048, C=1024) on 8 Trainium2 cores.

Sharding: core = 2*b + p handles batch b and two 512-row sequence blocks,
p=0 -> blocks {0, 3}, p=1 -> blocks {1, 2} (pairing balances causal work).
K/V projections for the full sequence are computed redundantly on both
cores of a batch pair; the causal skip of upper-triangle score/PV tiles
pays back exactly that duplication, so per-core FLOPs equal an ideal
8-way split (~17.2 GFLOP).

All matmul operands are bf16 (same 1 col/cycle PE rate as f32r but half
the DMA/SBUF/LDWEIGHTS traffic; measured rel err ~5e-3 vs the fp32
reference); accumulation is fp32 in PSUM, softmax denominator/reciprocal
and biases stay fp32.

On-device layout is transposed ([feature, token]) so every matmul
contracts along the partition axis:
    qT/kT = W.T @ xT                       (projection)
    scoresT[j, i] = kT_slice.T @ qT        (j on partitions)
    attnT[c, i]  += v_slice.T @ probsT     (accumulate over j tiles)
    outT          = Wo_slice.T @ attnT
Phase order is Q -> K/V -> attention; every weight is host-pre-laid-out
so its load is one contiguous DMA, and all loads are prefetched at t=0
across four engine queues (sync/gpsimd/scalar/vector), so the PE never
waits on HBM.  V stays resident in SBUF (no DRAM staging).  Softmax is
unnormalized (scores ~ N(0,1), exp is safe); the denominator comes from
an M=1 ones-column matmul over the masked exp tiles, its reciprocal is
broadcast to 128 partitions with a K=1 ones matmul, and applied to attnT
at the PSUM->SBUF copy.  Causal masks arrive as per-core input data (a
shifted window into a master 0/1 pattern), so all 8 cores run one SPMD
program even though their absolute row offsets differ.
"""

import math

import numpy as np

P = 128          # SBUF partitions
B_, T_, C_ = 4, 2048, 1024


def _emit(nc, tc, aps, T, C):
    from concourse import bass_isa, mybir
    from concourse.tile_rust import add_dep_helper
    from contextlib import ExitStack

    AFT = mybir.ActivationFunctionType
    f32 = mybir.dt.float32
    bf16 = mybir.dt.bfloat16

    NT = C // P            # feature tiles (8)
    BLK = T // 4           # sequence block = i-slot width FB (512)
    TL = 2 * BLK           # local query tokens per core (1024)
    FB = BLK               # matmul moving free dim for i (512)
    FC = 512               # x chunk width (Q and K/V passes)
    NCHK = T // FC         # 4 K/V chunks
    NQC = TL // FC         # 2 Q chunks
    NJ = T // P            # 16 total j-tiles of v
    njA = (2 * BLK) // P   # padded j-tiles for slot A (8)
    njB = (4 * BLK) // P   # padded j-tiles for slot B (16)
    CA = P * (njA - 1)
    NJB0 = njA            # slot-B j-tiles < NJB0 are causally all-ones for
    CB = P * (njB - 1 - NJB0)   # every core; mask multiply is skipped there
    SC = 1.0 / math.sqrt(C)

    (xR, xqR, WqR, WkR, WvR, WoR, bq_t, bk_t, bo_t, mA, mB, outT) = aps

    with ExitStack() as ctx:
        singles = ctx.enter_context(tc.tile_pool(name="singles", bufs=1))
        kpool = ctx.enter_context(tc.tile_pool(name="kpool", bufs=1))
        qpool = ctx.enter_context(tc.tile_pool(name="qpool", bufs=1))
        vpool = ctx.enter_context(tc.tile_pool(name="vpool", bufs=1))
        wpool = ctx.enter_context(tc.tile_pool(name="wpool", bufs=1))
        maskp = ctx.enter_context(tc.tile_pool(name="maskp", bufs=1))

        # ---- prefetch everything up front, spread across engine queues ----
        bq_sb = singles.tile([P, NT], f32, name="bq_sb")
        bk_sb = singles.tile([P, NT], f32, name="bk_sb")
        bo_sb = singles.tile([P, NT], f32, name="bo_sb")
        nc.scalar.dma_start(out=bq_sb, in_=bq_t)
        nc.scalar.dma_start(out=bk_sb, in_=bk_t)
        nc.scalar.dma_start(out=bo_sb, in_=bo_t)

        # weights, one contiguous panel per co on gpsimd/scalar queues
        wk_sb = wpool.tile([P, NT, NT, P], bf16, name="wk_sb")
        wv_sb = wpool.tile([P, NT, C], bf16, name="wv_sb")
        wo_sb = wpool.tile([P, NT, NT, P], bf16, name="wo_sb")

        mA_sb = maskp.tile([P, CA + FB], bf16, name="mA_sb")
        mB_sb = maskp.tile([P, CB + FB], bf16, name="mB_sb")

        kT_sb = kpool.tile([P, NT, T], bf16, name="kT_sb")
        qT_sb = qpool.tile([P, NT, TL], bf16, name="qT_sb")
        v_sb = vpool.tile([P, NJ, C], bf16, name="v_sb")

        # x chunks: xq first, then the 4 K/V chunks, all on the sync queue
        with ExitStack() as p1:
            wqpool = p1.enter_context(tc.tile_pool(name="wqpool", bufs=1))
            xpool = p1.enter_context(tc.tile_pool(name="xpool", bufs=3))
            psp1 = p1.enter_context(
                tc.tile_pool(name="psp1", bufs=6, space="PSUM"))
            wq_sb = wqpool.tile([P, NT, NT, P], bf16, name="wq_sb")
            # gpsimd queue: wq panels first (Q consumes them at 1.7us each),
            # then wk, then masks.  sync queue: xq0 as per-ci strips (matmul
            # 0 only waits on ~160KB), xq1, the K/V x chunks, then wv/wo
            # (needed only from ~45us/~200us).  scalar queue stays free for
            # the activation stream.
            for co in range(NT):
                nc.gpsimd.dma_start(out=wq_sb[:, co], in_=WqR[:, co])
            for co in range(NT):
                nc.gpsimd.dma_start(out=wk_sb[:, co], in_=WkR[:, co])
            nc.gpsimd.dma_start(out=mA_sb, in_=mA)
            nc.gpsimd.dma_start(out=mB_sb, in_=mB)
            xqs = []
            for qc in range(NQC):
                xcq = xpool.tile([P, NT, FC], bf16, name="xcq", tag="xc")
                if qc == 0:
                    for ci in range(NT):
                        nc.sync.dma_start(out=xcq[:, ci], in_=xqR[qc, :, ci])
                else:
                    nc.sync.dma_start(out=xcq, in_=xqR[qc])
                xqs.append(xcq)

            # ---- pass Q: qT = Wq.T @ xq (+bq) ----
            for qc in range(NQC):
                for co in range(NT):
                    ps = psp1.tile([P, FC], f32, name="ps_q", tag="ps")
                    for ci in range(NT):
                        nc.tensor.matmul(
                            ps,
                            wq_sb[:, co, ci, :],
                            xqs[qc][:, ci, :],
                            start=(ci == 0),
                            stop=(ci == NT - 1),
                        )
                    nc.scalar.activation(
                        out=qT_sb[:, co, qc * FC:(qc + 1) * FC],
                        in_=ps,
                        func=AFT.Identity,
                        bias=bq_sb[:, co:co + 1],
                    )

            nc.sync.dma_start(out=wv_sb, in_=WvR)
            nc.sync.dma_start(out=wo_sb, in_=WoR)

            # ---- pass K+V per x chunk ----
            for jc in range(NCHK):
                xck = xpool.tile([P, NT, FC], bf16, name="xck", tag="xc")
                nc.sync.dma_start(out=xck, in_=xR[jc])
                for co in range(NT):
                    ps = psp1.tile([P, FC], f32, name="ps_k", tag="ps")
                    for ci in range(NT):
                        nc.tensor.matmul(
                            ps,
                            wk_sb[:, co, ci, :],
                            xck[:, ci, :],
                            start=(ci == 0),
                            stop=(ci == NT - 1),
                        )
                    nc.scalar.activation(
                        out=kT_sb[:, co, jc * FC:(jc + 1) * FC],
                        in_=ps,
                        func=AFT.Identity,
                        bias=bk_sb[:, co:co + 1],
                    )
                # v = x @ Wv  (bv folded into bo_t on host)
                for jt in range(FC // P):
                    for ch in range(2):
                        ps = psp1.tile([P, 512], f32, name="ps_v", tag="ps")
                        for ci in range(NT):
                            nc.tensor.matmul(
                                ps,
                                xck[:, ci, jt * P:(jt + 1) * P],
                                wv_sb[:, ci, ch * 512:(ch + 1) * 512],
                                start=(ci == 0),
                                stop=(ci == NT - 1),
                            )
                        nc.vector.tensor_copy(
                            v_sb[:, jc * (FC // P) + jt,
                                 ch * 512:(ch + 1) * 512],
                            ps,
                        )

        # -------- phase 2: attention + output projection --------
        with ExitStack() as p2:
            probsp = p2.enter_context(tc.tile_pool(name="probsp", bufs=njA + njB))
            attnp = p2.enter_context(tc.tile_pool(name="attnp", bufs=2))
            recp = p2.enter_context(tc.tile_pool(name="recp", bufs=2))
            ostagep = p2.enter_context(tc.tile_pool(name="ostagep", bufs=2))
            pscore = p2.enter_context(
                tc.tile_pool(name="pscore", bufs=2, space="PSUM"))
            pattn = p2.enter_context(
                tc.tile_pool(name="pattn", bufs=4, space="PSUM"))
            pout = p2.enter_context(
                tc.tile_pool(name="pout", bufs=2, space="PSUM"))

            attns = []
            for a, (nj, j0m, Cm, m_sb) in enumerate(
                [(njA, 0, CA, mA_sb), (njB, NJB0, CB, mB_sb)]
            ):
                # scores + exp + mask; the denominator accumulates on the
                # vector engine (fp32) instead of PE ones-matmuls
                probs_tiles = []
                dacc = recp.tile([P, FB], f32, name="dacc", tag="dacc")
                for jt in range(nj):
                    ps_s = pscore.tile([P, FB], f32, name="ps_s", tag="ps_s")
                    for ci in range(NT):
                        nc.tensor.matmul(
                            ps_s,
                            kT_sb[:, ci, jt * P:(jt + 1) * P],
                            qT_sb[:, ci, a * FB:(a + 1) * FB],
                            start=(ci == 0),
                            stop=(ci == NT - 1),
                        )
                    pj = probsp.tile([P, FB], bf16, name="pj", tag="pj")
                    nc.scalar.activation(out=pj, in_=ps_s, func=AFT.Exp, scale=SC)
                    if jt >= j0m:  # earlier j-tiles are all-ones on every core
                        s0 = Cm - P * (jt - j0m)
                        nc.vector.tensor_mul(pj, pj, m_sb[:, s0:s0 + FB])
                    if jt == 0:
                        nc.vector.tensor_copy(dacc, pj)
                    else:
                        nc.vector.tensor_add(dacc, dacc, pj)
                    probs_tiles.append(pj)

                # 1/denominator: gpsimd all-reduces the per-partition sums
                # (broadcast to all partitions), fast approx reciprocal on
                # vector; everything off the PE queue
                den_f = recp.tile([P, FB], f32, name="den_f", tag="den_f")
                nc.gpsimd.partition_all_reduce(
                    den_f, dacc, channels=P, reduce_op=bass_isa.ReduceOp.add)
                rec_sb = recp.tile([P, FB], f32, name="rec_sb", tag="rec_sb")
                nc.vector.reciprocal_approx_fast(rec_sb, den_f)

                # PV: attnT[c, i] accumulated over j tiles, four banks at a
                # time (2 passes over the probs tiles) so slot B scores can
                # overlap slot A PV on the free score banks
                attn_sb = attnp.tile([P, NT, FB], bf16, name="attn_sb",
                                     tag="attn")
                for half in range(2):
                    ps_attn = [
                        pattn.tile([P, FB], f32, name="ps_attn", tag="ps_a")
                        for _ in range(4)
                    ]
                    for c4 in range(4):
                        ct = half * 4 + c4
                        for jt in range(nj):
                            nc.tensor.matmul(
                                ps_attn[c4],
                                v_sb[:, jt, ct * P:(ct + 1) * P],
                                probs_tiles[jt],
                                start=(jt == 0),
                                stop=(jt == nj - 1),
                                skip_group_check=True,
                            )
                    for c4 in range(4):
                        ct = half * 4 + c4
                        nc.vector.tensor_mul(
                            attn_sb[:, ct, :], ps_attn[c4], rec_sb)
                attns.append(attn_sb)

            # output projections last (+ folded bv@Wo + bo bias): slot A's
            # runs while slot B's attn muls finish, so the PE never waits
            for a in range(2):
                attn_sb = attns[a]
                for co in range(NT):
                    ps_o = pout.tile([P, FB], f32, name="ps_o", tag="ps_o")
                    for ci in range(NT):
                        nc.tensor.matmul(
                            ps_o,
                            wo_sb[:, co, ci, :],
                            attn_sb[:, ci, :],
                            start=(ci == 0),
                            stop=(ci == NT - 1),
                        )
                    os_ = ostagep.tile([P, FB], bf16, name="os_", tag="os")
                    nc.scalar.activation(
                        out=os_, in_=ps_o, func=AFT.Identity,
                        bias=bo_sb[:, co:co + 1],
                    )
                    nc.sync.dma_start(
                        out=outT[co * P:(co + 1) * P, a * FB:(a + 1) * FB],
                        in_=os_,
                    )


def build_program(T=T_, C=C_, num_cores=8):
    """Build and compile the SPMD Bass program."""
    from concourse import bacc, mybir
    import concourse.tile as tile

    f32 = mybir.dt.float32
    bf16 = mybir.dt.bfloat16
    NT = C // P
    BLK = T // 4
    TL = 2 * BLK
    FC = 512
    njA = (2 * BLK) // P
    njB = (4 * BLK) // P
    CA = P * (njA - 1)
    CB = P * (njB - 1 - njA)

    nc = bacc.Bacc(
        "TRN2", target_bir_lowering=False, debug=False, num_devices=num_cores
    )
    xR = nc.dram_tensor("xR", [T // FC, P, NT, FC], bf16,
                        kind="ExternalInput").ap()
    xqR = nc.dram_tensor("xqR", [TL // FC, P, NT, FC], bf16,
                         kind="ExternalInput").ap()
    WqR = nc.dram_tensor("WqR", [P, NT, NT, P], bf16, kind="ExternalInput").ap()
    WkR = nc.dram_tensor("WkR", [P, NT, NT, P], bf16, kind="ExternalInput").ap()
    WvR = nc.dram_tensor("WvR", [P, NT, C], bf16, kind="ExternalInput").ap()
    WoR = nc.dram_tensor("WoR", [P, NT, NT, P], bf16, kind="ExternalInput").ap()
    bq_t = nc.dram_tensor("bq_t", [P, NT], f32, kind="ExternalInput").ap()
    bk_t = nc.dram_tensor("bk_t", [P, NT], f32, kind="ExternalInput").ap()
    bo_t = nc.dram_tensor("bo_t", [P, NT], f32, kind="ExternalInput").ap()
    mA = nc.dram_tensor("mA", [P, CA + BLK], bf16, kind="ExternalInput").ap()
    mB = nc.dram_tensor("mB", [P, CB + BLK], bf16, kind="ExternalInput").ap()
    outT = nc.dram_tensor("outT", [C, TL], bf16, kind="ExternalOutput").ap()

    aps = (xR, xqR, WqR, WkR, WvR, WoR, bq_t, bk_t, bo_t, mA, mB, outT)
    with tile.TileContext(nc) as tc:
        _emit(nc, tc, aps, T, C)
    nc.compile()
    return nc


def make_core_inputs(x, Wq, bq, Wk, bk, Wv, bv, Wo, bo, T=T_, C=C_):
    """Per-core input maps (list of 8 dicts) for the SPMD program."""
    import ml_dtypes

    f = np.float32
    b16 = ml_dtypes.bfloat16
    NT = C // P
    BLK = T // 4
    FC = 512
    njA = (2 * BLK) // P
    njB = (4 * BLK) // P
    CA = P * (njA - 1)
    CB = P * (njB - 1 - njA)

    x = np.asarray(x, f)
    Wq, Wk, Wv, Wo = (np.asarray(w, f) for w in (Wq, Wk, Wv, Wo))
    bq, bk, bv, bo = (np.asarray(b, f) for b in (bq, bk, bv, bo))

    def cotile(W):  # [C, C] -> [P, NT(co), NT(ci), P]: W[ci*P+p, co*P+m]
        return np.ascontiguousarray(
            W.reshape(NT, P, NT, P).transpose(1, 2, 0, 3)).astype(b16)

    WqRl = cotile(Wq)
    WkRl = cotile(Wk)
    WoRl = cotile(Wo)
    WvRl = np.ascontiguousarray(Wv.reshape(NT, P, C).transpose(1, 0, 2)
                                ).astype(b16)
    bo_eff = (bv @ Wo + bo).astype(f)

    def tr(b):  # [C] -> [P, NT] with b_t[p, t] = b[t*P + p]
        return np.ascontiguousarray(b.reshape(NT, P).T)

    def mask(CC, i0, width):
        pp = np.arange(P, dtype=np.int64)[:, None]
        gg = np.arange(width, dtype=np.int64)[None, :]
        return np.ascontiguousarray((pp <= gg - CC + i0).astype(b16))


    def chunked(xT):  # [C, W] -> [W//FC, P, NT, FC]: xT[ci*P+p, c*FC+t]
        W = xT.shape[1]
        return np.ascontiguousarray(
            xT.reshape(NT, P, W // FC, FC).transpose(2, 1, 0, 3)).astype(b16)

    maps = []
    for core in range(8):
        b, p = core // 2, core % 2
        lo, hi = (0, 3) if p == 0 else (1, 2)
        xTv = np.ascontiguousarray(x[b].T)  # [C, T]
        xqb = np.concatenate(
            [xTv[:, lo * BLK:(lo + 1) * BLK], xTv[:, hi * BLK:(hi + 1) * BLK]],
            axis=1,
        )
        maps.append(
            {
                "xR": chunked(xTv),
                "xqR": chunked(xqb),
                "WqR": WqRl,
                "WkR": WkRl,
                "WvR": WvRl,
                "WoR": WoRl,
                "bq_t": tr(bq),
                "bk_t": tr(bk),
                "bo_t": tr(bo_eff),
                "mA": mask(CA, lo * BLK, CA + BLK),
                "mB": mask(CB + njA * P, hi * BLK, CB + BLK),
            }
        )
    return maps


def gather_output(results, T=T_, C=C_, B=B_):
    BLK = T // 4
    out = np.empty((B, T, C), np.float32)
    for core in range(8):
        b, p = core // 2, core % 2
        lo, hi = (0, 3) if p == 0 else (1, 2)
        oT = np.asarray(results[core]["outT"], np.float32)
        out[b, lo * BLK:(lo + 1) * BLK] = oT[:, 0:BLK].T
        out[b, hi * BLK:(hi + 1) * BLK] = oT[:, BLK:2 * BLK].T
    return out


_NC_CACHE = {}


def kernel(x, Wq, bq, Wk, bk, Wv, bv, Wo, bo):
    from concourse.bass_utils import run_bass_kernel_spmd

    key = "full"
    if key not in _NC_CACHE:
        _NC_CACHE[key] = build_program()
    nc = _NC_CACHE[key]
    in_maps = make_core_inputs(x, Wq, bq, Wk, bk, Wv, bv, Wo, bo)
    res = run_bass_kernel_spmd(nc, in_maps, list(range(8))).results
    return gather_output(res)
